# revision 27
# baseline (speedup 1.0000x reference)
"""Trainium2 Bass kernel for Felsenstein pruning on a perfect binary tree
(nn_BaseLikelihoodModel: batched expm over edges + level-synchronous sweep).

Every edge matrix is expm(t_e * R) for ONE shared 16x16 rate matrix
R = Q - diag(growth).  A real block-eigendecomposition R = W M W^-1
(host-side, f64) turns the per-edge expm into per-edge exp/sin/cos factors
plus shared 16x16 matmuls:

    expm(t R) v = W * B(t) * (W^-1 v)

Key structure (v3):
  * swap-fold: Winv[swap] = P Winv and pairs share Re(lambda) with opposite
    Im signs, so swap_rows(ES) = -ES and the second matvec chain collapses:
    Y = W (EC * U) + W[:,swap] ((-ES) * U) with U = Winv V.
  * sin/cos via short polynomials on DVE/GpSimd (|b t| <= ~0.35); Scalar
    only ever needs the exp+ln activation table (no table thrash).
  * per-edge argument x = b*t is produced by a second PE broadcast with
    b-scaled selection weights; all inputs ship as ONE packed fp16 tensor
    (t is fp16: |a|*t*5e-4 per-edge log error, random-walks to ~2e-6).
  * leaves enter as state indices; one-hots built via broadcast + is_equal.
  * per level, the parent combine Ye*(g*Yo) splits the Y matmul into
    even(left)/odd(right) column groups so the PSUM->SBUF copy of Ye
    overlaps the odd-column matmuls.
  * stacked [Winv;Winv] / [W | W_swap] weights for the narrow top levels;
    levels 13..16 replicated on every core after a tiny AllGather.

The sweep runs in probability space with one per-node rescaling at level 8
(accumulated log-scales), mathematically identical to the reference's
log-space logsumexp recursion.  Edge factors carry a constant boost
e^BOOST; the exact total (N-1)*BOOST is subtracted at the end.
"""
import math
import numpy as np
import ml_dtypes

import concourse.bass as bass
import concourse.mybir as mybir
import concourse.tile as tile
from concourse.bass_utils import run_bass_kernel_spmd

F32 = np.float32
F16 = np.float16
BF16 = ml_dtypes.bfloat16
S = 16
L = 32768
N = 2 * L
NCORES = 8
NBLK = 8
LPC = L // NCORES          # 4096 leaves per core
LPB = LPC // NBLK          # 512 leaves per block
BLK_W = [LPB >> hc for hc in range(9)]             # 512..2
BLK_OFF = np.concatenate([[0], np.cumsum(BLK_W)])  # offsets into 1022
BLK_TOTAL = int(BLK_OFF[-1])                       # 1022
# top-edge columns TOPO..TOPO+28:
# [lvl9 x8][lvl10 x4][lvl11 x2][lvl12 x8][lvl13 x4][lvl14 x2][lvl15 x1]
TOPO = BLK_TOTAL                                   # 1022
T_ALL = 1056                                       # padded total columns
CHUNKS = [(0, 512), (512, 512), (1024, 32)]        # PSUM-bank sized chunks

OFFS = [0]
for _h in range(1, 16):
    OFFS.append(OFFS[-1] + (L >> (_h - 1)))
ROOT = N - 1

BOOST = 1.7
CORR = float(np.float64(N - 1) * np.float64(np.float32(BOOST)))
AGW = 32  # AllGather payload per core (16 state values + logscale + pad)

# pack16 column layout (fp16 [8, 1824])
P16_EXPD = 0      # kron(I8, ones(1,16))              [8, 0:128]
P16_EXPB = 128    # kron(I8, bsig row)                [8, 128:256]
P16_T = 256       # t_blk                             [8, 256:1312]
P16_SID = 1312    # leaf state ids                    [8, 1312:1824]
P16_COLS = 1824
# packbf column layout (bf16 [128, 584])
PB_WINV = 0      # kron(I8, Winv.T)            [128, 0:128]
PB_W = 128       # kron(I8, W.T)               [128, 128:256]
PB_WSW = 256     # kron(I8, W[:,swap].T)       [128, 256:384]
PB_ONESBD = 384  # kron(I8, ones(16,16))       [128, 384:512]
PB_ONESC = 512   # kron(I8, ones(16,1))        [128, 512:520]
PB_ITILE = 520   # tile(eye(16), (8,1))        [128, 520:536]
PB_U2 = 536      # [Winv.T | Winv.T]           [0:16, 536:568]
PB_Y2 = 568      # [W.T ; W[:,swap].T]         [0:32, 568:584]
PB_COLS = 584
# packf column layout (f32 [128, 52])
PF_AVEC = 0
PF_IOTA = 2
PF_GCOL = 3
PF_ONESF = 4     # ones row-0                  [0:1, 4:20]
PF_U2F = 20      # [Winv.T | Winv.T] f32       [0:16, 20:52]
PF_COLS = 52

# sin/cos polynomial coefficients (|x| <= ~0.35); sine coeffs negated so the
# chain directly yields -sin(x) (the swap-fold sign).
NS5 = float(-1.0 / 120.0)
NS3 = float(1.0 / 6.0)
NS1 = float(-1.0)
C4 = float(1.0 / 24.0)
C2 = float(-0.5)


def _real_eig(R):
    """Real block eigendecomposition R = Wr @ M @ inv(Wr) with M block
    diagonal ([[a, b], [-b, a]] blocks for conjugate pairs)."""
    ev, V = np.linalg.eig(R)
    used = np.zeros(S, bool)
    order = np.argsort(-ev.real)
    cols = []
    for i in order:
        if used[i]:
            continue
        if abs(ev[i].imag) < 1e-12:
            cols.append(("real", i))
            used[i] = True
        else:
            j = None
            for i2 in order:
                if not used[i2] and i2 != i and abs(ev[i2] - ev[i].conj()) < 1e-8:
                    j = i2
                    break
            assert j is not None, "unpaired complex eigenvalue"
            ip = i if ev[i].imag > 0 else j
            cols.append(("pair", ip))
            used[i] = used[j] = True
    Wr = np.zeros((S, S))
    a = np.zeros(S)
    bsig = np.zeros(S)
    swap = np.arange(S)
    k = 0
    for c in cols:
        if c[0] == "real":
            i = c[1]
            Wr[:, k] = V[:, i].real
            a[k] = ev[i].real
            k += 1
        else:
            ip = c[1]
            lam = ev[ip]
            Wr[:, k] = V[:, ip].real
            Wr[:, k + 1] = V[:, ip].imag
            a[k] = a[k + 1] = lam.real
            bsig[k] = lam.imag
            bsig[k + 1] = -lam.imag
            swap[k] = k + 1
            swap[k + 1] = k
            k += 2
    assert k == S
    scales = np.ones(S)
    kk = 0
    while kk < S:
        if swap[kk] == kk:
            scales[kk] = np.linalg.norm(Wr[:, kk])
            kk += 1
        else:
            s = math.sqrt(np.linalg.norm(Wr[:, kk]) * np.linalg.norm(Wr[:, kk + 1]))
            scales[kk] = scales[kk + 1] = s
            kk += 2
    Wr = Wr / scales[None, :]
    Winv = np.linalg.inv(Wr)
    return Wr, Winv, a, bsig, swap


def _split_multi_waits(nc):
    """Walrus codegen allows only ONE sync-wait slot per engine instruction.
    Move extra waits onto prepended same-engine NoOps (queue order stalls
    identically)."""
    skip = (mybir.InstAllEngineBarrier, mybir.InstBranchHint,
            mybir.InstCompareAndBranch, mybir.InstUnconditionalBranch,
            mybir.InstIndirectBranch)
    for fn in nc.m.functions:
        for blk in fn.blocks:
            out = []
            for inst in blk.instructions:
                si = inst.sync_info
                if (si is not None and si.on_wait and len(si.on_wait) > 1
                        and not isinstance(inst, skip)):
                    waits = list(si.on_wait)
                    for i, w in enumerate(waits[:-1]):
                        nop = mybir.InstNoOp(
                            name=f"{inst.name}-wait{i}", ins=[], outs=[])
                        nop.engine = inst.engine
                        nop.sync_info = mybir.SyncInfo(
                            on_wait=[w], on_update=[])
                        out.append(nop)
                    inst.sync_info = mybir.SyncInfo(
                        on_wait=[waits[-1]], on_update=list(si.on_update or []))
                out.append(inst)
            blk.instructions = out


def build_nc(split_waits=True):
    f32 = mybir.dt.float32
    bf16 = mybir.dt.bfloat16
    f16 = mybir.dt.float16
    AF = mybir.ActivationFunctionType
    OP = mybir.AluOpType
    nc = bass.Bass()

    pack16 = nc.dram_tensor("pack16", [8, P16_COLS], f16, kind="ExternalInput")
    packbf = nc.dram_tensor("packbf", [128, PB_COLS], bf16, kind="ExternalInput")
    packf = nc.dram_tensor("packf", [128, PF_COLS], f32, kind="ExternalInput")
    out = nc.dram_tensor("out", [16, 1], f32, kind="ExternalOutput")
    agin = nc.dram_tensor("agin", [AGW, 1], f32)
    agout = nc.dram_tensor("agout", [NCORES * AGW, 1], f32, addr_space="Shared")

    with tile.TileContext(nc) as tc:
        with (
            tc.tile_pool(name="const", bufs=1) as cp,
            tc.tile_pool(name="sb", bufs=2) as sb,
            tc.tile_pool(name="big", bufs=1) as bigp,
            tc.tile_pool(name="psA", bufs=2, space="PSUM") as psA,
            tc.tile_pool(name="psU", bufs=2, space="PSUM") as psU,
            tc.tile_pool(name="psY", bufs=1, space="PSUM") as psY,
            tc.tile_pool(name="psN", bufs=1, space="PSUM") as psN,
        ):
            # ---- input DMAs, issued from different engines so the ~0.8us
            # DIRECT2D issue cost doesn't serialize on one sequencer
            s_p16 = cp.tile([8, P16_COLS], f16, tag="p16")
            nc.scalar.dma_start(s_p16[:], pack16[:, :])
            s_pf = cp.tile([128, PF_COLS], f32, tag="packf")
            nc.gpsimd.dma_start(s_pf[:], packf[:, :])
            s_pb = cp.tile([128, PB_COLS], bf16, tag="packbf")
            nc.sync.dma_start(s_pb[:], packbf[:, :])

            v_expd = s_p16[:, P16_EXPD:P16_EXPD + 128]
            v_expB = s_p16[:, P16_EXPB:P16_EXPB + 128]
            v_t = s_p16[:, P16_T:P16_T + T_ALL]
            v_sid = s_p16[:, P16_SID:P16_SID + 512]
            c_winvT = s_pb[:, PB_WINV:PB_WINV + 128]
            c_wT = s_pb[:, PB_W:PB_W + 128]
            c_wswT = s_pb[:, PB_WSW:PB_WSW + 128]
            c_onesbd = s_pb[:, PB_ONESBD:PB_ONESBD + 128]
            c_onesc = s_pb[:, PB_ONESC:PB_ONESC + 8]
            c_itile = s_pb[:, PB_ITILE:PB_ITILE + 16]
            c_u2T = s_pb[0:16, PB_U2:PB_U2 + 32]
            c_y2T = s_pb[0:32, PB_Y2:PB_Y2 + 16]
            c_avec = s_pf[:, PF_AVEC:PF_AVEC + 1]
            c_iota = s_pf[:, PF_IOTA:PF_IOTA + 1]
            c_gcol = s_pf[:, PF_GCOL:PF_GCOL + 1]
            c_onesf = s_pf[0:1, PF_ONESF:PF_ONESF + 16]
            c_u2Tf = s_pf[0:16, PF_U2F:PF_U2F + 32]

            cBOOST = cp.tile([128, 1], f32, tag="boost")
            nc.vector.memset(cBOOST[:], float(BOOST))
            ones8 = cp.tile([8, 1], f32, tag="ones8")
            nc.vector.memset(ones8[:], 1.0)
            ls12 = cp.tile([16, 1], f32, tag="ls12")
            nc.vector.memset(ls12[:], 0.0)
            # dummy activation with no data dependency: forces the exp+ln
            # ACT table load at engine boot instead of on the critical path
            dummy = cp.tile([1, 1], f32, tag="dummy")
            nc.scalar.activation(dummy[:], cBOOST[0:1, 0:1], AF.Exp)

            # ---- PE queue-observer preamble (one per DMA'd matmul operand)
            pobs = psY.tile([1, 1], f32, tag="Ye")
            nc.tensor.matmul(pobs[:], s_p16[0:1, 0:1], s_p16[0:1, 0:1],
                             start=True, stop=True)
            pobs2 = psN.tile([1, 1], f32, tag="N")
            nc.tensor.matmul(pobs2[:], s_pb[0:1, 0:1], s_pb[0:1, 0:1],
                             start=True, stop=True)
            pobs3 = psN.tile([1, 1], f32, tag="N2")
            nc.tensor.matmul(pobs3[:], s_pf[0:1, 0:1], s_pf[0:1, 0:1],
                             start=True, stop=True)

            # ---- leaf one-hots: broadcast state ids, compare to row state
            pSID = psA.tile([128, 512], f32, tag="T")
            nc.tensor.matmul(pSID[:], v_expd, v_sid, start=True, stop=True)
            sX = bigp.tile([128, 512], bf16, tag="V0")
            nc.vector.tensor_scalar(sX[:], pSID[:], c_iota, None, OP.is_equal)

            # ---- edge factors.  Per chunk: T128 = t broadcast (PE), x = b*t
            # broadcast (PE, b-scaled weights); E = exp(a t + BOOST) and
            # x^2 (Square) on Scalar; -sin/cos chains in bf16 on DVE/GpSimd.
            # EC and ESp live in ONE tile so the per-level m12 multiply can
            # read [EC_lvl ; ESp_lvl] as a single strided AP.
            ECES = bigp.tile([128, 2 * T_ALL], bf16, tag="ECES")
            EC = ECES[:, 0:T_ALL]
            ESp = ECES[:, T_ALL:2 * T_ALL]
            sE = bigp.tile([128, T_ALL], bf16, tag="sE")
            for lo, wch in CHUNKS:
                pT = psA.tile([128, wch], f32, tag="T")
                nc.tensor.matmul(pT[:], v_expd, v_t[:, lo:lo + wch],
                                 start=True, stop=True)
                pX = psA.tile([128, wch], f32, tag="T")
                nc.tensor.matmul(pX[:], v_expB, v_t[:, lo:lo + wch],
                                 start=True, stop=True)
                nc.scalar.activation(sE[:, lo:lo + wch], pT[:], AF.Exp,
                                     bias=cBOOST[:, 0:1], scale=c_avec)
                x2 = sb.tile([128, wch], bf16, tag="x2")
                nc.scalar.activation(x2[:], pX[:], AF.Square)
                # 2-term -sin: (x2/6 - 1) * x   (err x^5/120, << bf16 noise)
                q = sb.tile([128, wch], bf16, tag="q")
                nc.vector.tensor_scalar(q[:], x2[:], float(1.0 / 6.0), -1.0,
                                        OP.mult, OP.add)
                r = sb.tile([128, wch], bf16, tag="r")
                nc.vector.tensor_mul(r[:], q[:], pX[:])
                if lo == 0:
                    nc.vector.tensor_mul(ESp[:, lo:lo + wch], r[:],
                                         sE[:, lo:lo + wch])
                else:
                    nc.gpsimd.tensor_mul(ESp[:, lo:lo + wch], r[:],
                                         sE[:, lo:lo + wch])
                # 2-term cos * E: E + (x2 * -0.5) * E   (err x^4/24)
                p = sb.tile([128, wch], bf16, tag="p")
                nc.vector.scalar_tensor_tensor(
                    p[:], x2[:], -0.5, sE[:, lo:lo + wch], OP.mult, OP.mult)
                if lo == 0:
                    nc.vector.tensor_add(EC[:, lo:lo + wch],
                                         sE[:, lo:lo + wch], p[:])
                else:
                    nc.gpsimd.tensor_add(EC[:, lo:lo + wch],
                                         sE[:, lo:lo + wch], p[:])
            # stacked top-edge factors [EC; ESp] (both from block 0); the
            # SBUF->SBUF DMA sidesteps the 32-aligned partition-base rule
            ECS = sb.tile([32, 29], bf16, tag="ECS")
            nc.sync.dma_start(ECS[0:16, :], EC[0:16, TOPO:TOPO + 29])
            nc.sync.dma_start(ECS[16:32, :], ESp[0:16, TOPO:TOPO + 29])

            # ---- one sweep level: U = Winv V; m12 = [EC_lvl|ESp_lvl] * [U|U]
            # (one DVE op via a 3-dim AP into ECES); Ye/Yo = even/odd column
            # groups of W m1 + Wsw m2 so the Ye copy overlaps the odd
            # matmuls; parent = Ye * (g * Yo).
            ecs3 = ECES[:].rearrange("p (k w) -> p k w", k=2)

            def level(V, lo, wc, h):
                wp = wc // 2
                if wc > 256:
                    # 2*wc f32 would cross a PSUM bank; use the 2-op form
                    pU = psU.tile([128, wc], f32, tag="U")
                    nc.tensor.matmul(pU[:], c_winvT, V, start=True, stop=True)
                    m12 = sb.tile([128, 2 * wc], bf16, tag="m12w")
                    nc.vector.tensor_mul(m12[:, 0:wc], EC[:, lo:lo + wc], pU[:])
                    nc.vector.tensor_mul(m12[:, wc:2 * wc],
                                         ESp[:, lo:lo + wc], pU[:])
                else:
                    pU = psU.tile([128, 2 * wc], f32, tag="U")
                    nc.tensor.matmul(pU[:, 0:wc], c_winvT, V,
                                     start=True, stop=True)
                    nc.tensor.matmul(pU[:, wc:2 * wc], c_winvT, V,
                                     start=True, stop=True)
                    m12 = sb.tile([128, 2 * wc], bf16, tag="m12w")
                    nc.vector.tensor_mul(
                        m12[:].rearrange("p (k w) -> p k w", k=2),
                        ecs3[:, :, lo:lo + wc],
                        pU[:].rearrange("p (k w) -> p k w", k=2))
                pYe = psY.tile([128, wp], f32, tag="Ye")
                nc.tensor.matmul(pYe[:], c_wT, m12[:, 0:wc:2],
                                 start=True, stop=False)
                nc.tensor.matmul(pYe[:], c_wswT, m12[:, wc:2 * wc:2],
                                 start=False, stop=True)
                sYe = sb.tile([128, wp], f32, tag="sYe")
                if h <= 2:
                    nc.scalar.copy(sYe[:], pYe[:])
                else:
                    nc.vector.tensor_copy(sYe[:], pYe[:])
                pYo = psY.tile([128, wp], f32, tag="Yo")
                nc.tensor.matmul(pYo[:], c_wT, m12[:, 1:wc:2],
                                 start=True, stop=False)
                nc.tensor.matmul(pYo[:], c_wswT, m12[:, wc + 1:2 * wc:2],
                                 start=False, stop=True)
                return pYo, sYe

            # ---- level sweep 1..9 (8 blocks x 16 states on 128 partitions)
            V = sX
            lsW = None
            for h in range(1, 10):
                wc = BLK_W[h - 1]
                lo = int(BLK_OFF[h - 1])
                wp = wc // 2
                pYo, sYe = level(V[:], lo, wc, h)
                if h == 8:
                    praw = sb.tile([128, wp], bf16, tag="Vc")
                    nc.vector.scalar_tensor_tensor(
                        praw[:], pYo[:], c_gcol, sYe[:], OP.mult, OP.mult)
                    pSb = psN.tile([128, wp], f32, tag="N")
                    nc.tensor.matmul(pSb[:], c_onesbd, praw[:],
                                     start=True, stop=True)
                    pSc = psN.tile([8, wp], f32, tag="N2")
                    nc.tensor.matmul(pSc[:], c_onesc, praw[:],
                                     start=True, stop=True)
                    rb = sb.tile([128, wp], f32, tag="rb")
                    nc.vector.reciprocal(rb[:], pSb[:])
                    Vn = sb.tile([128, wp], bf16, tag="V")
                    nc.vector.tensor_mul(Vn[:], praw[:], rb[:])
                    lnS = sb.tile([8, wp], f32, tag="lnS")
                    nc.scalar.activation(lnS[:], pSc[:], AF.Ln)
                    lsW = lnS
                else:
                    Vn = sb.tile([128, wp], f32 if h == 9 else bf16, tag="V")
                    nc.vector.scalar_tensor_tensor(
                        Vn[:], pYo[:], c_gcol, sYe[:], OP.mult, OP.mult)
                V = Vn

            # per-core log-scale total
            ls9 = sb.tile([8, 1], f32, tag="ls9")
            nc.gpsimd.tensor_add(ls9[:], lsW[:, 0:1], lsW[:, 1:2])
            pls = psN.tile([1, 1], f32, tag="N2")
            nc.tensor.matmul(pls[:], ls9[:], ones8[:], start=True, stop=True)
            nc.vector.tensor_copy(ls12[0:1, :], pls[:])

            # ---- reshape core state to single block: V (128x1) -> (16x8)
            rhs8 = sb.tile([128, 8], bf16, tag="rhs8")
            nc.vector.tensor_scalar_mul(rhs8[:], c_onesc, V[:, 0:1])
            pV9 = psY.tile([16, 8], f32, tag="Ye")
            nc.tensor.matmul(pV9[:], c_itile, rhs8[:], start=True, stop=True)
            sV = sb.tile([16, 8], bf16, tag="sV")
            nc.vector.tensor_copy(sV[:], pV9[:])

            # ---- one stacked top level (32 partitions)
            def top_level(rhsV, off, n, lhsU):
                pU2 = psU.tile([32, n], f32, tag="U")
                nc.tensor.matmul(pU2[:], lhsU, rhsV, start=True, stop=True)
                m12 = sb.tile([32, n], bf16, tag="m12")
                nc.vector.tensor_mul(m12[:], ECS[:, off:off + n], pU2[:])
                if n == 1:
                    pYt = psY.tile([16, 1], f32, tag="Ye")
                    nc.tensor.matmul(pYt[:], c_y2T, m12[:], start=True, stop=True)
                    return pYt, None
                pYe = psY.tile([16, n // 2], f32, tag="Ye")
                nc.tensor.matmul(pYe[:], c_y2T, m12[:, 0::2], start=True, stop=True)
                sYe = sb.tile([16, n // 2], f32, tag="sYe")
                nc.vector.tensor_copy(sYe[:], pYe[:])
                pYo = psY.tile([16, n // 2], f32, tag="Yo")
                nc.tensor.matmul(pYo[:], c_y2T, m12[:, 1::2], start=True, stop=True)
                return pYo, sYe

            def top_combine(pYo, sYe, n2, out_dt, out_ap=None):
                if out_ap is None:
                    Vn = sb.tile([16, n2], out_dt, tag="sV")
                    out_ap = Vn[:]
                else:
                    Vn = None
                nc.vector.scalar_tensor_tensor(
                    out_ap, pYo[:], c_gcol[0:16, 0:1], sYe[:],
                    OP.mult, OP.mult)
                return Vn

            # levels 10..12 (within-core top; ECS cols 0:8, 8:12, 12:14)
            off = 0
            n = 8
            for h in (10, 11, 12):
                pYo, sYe = top_level(sV[:], off, n, c_u2T)
                off += n
                n //= 2
                if h == 12:
                    sV12 = sb.tile([16, 1], f32, tag="sV12")
                    top_combine(pYo, sYe, 1, f32, out_ap=sV12[:])
                else:
                    sV = top_combine(pYo, sYe, n, bf16)

            # ---- AllGather of (16-vec, logscale) across the 8 cores
            nc.sync.dma_start(agin[0:16, 0:1], sV12[:])
            nc.sync.dma_start(agin[16:32, 0:1], ls12[:])
            nc.gpsimd.collective_compute(
                "AllGather",
                OP.bypass,
                replica_groups=[list(range(NCORES))],
                ins=[agin[:, :].opt()],
                outs=[agout[:, :].opt()],
            )
            ag2 = agout[:, 0].rearrange("(r v) -> v r", v=AGW)
            sG = sb.tile([16, 8], f32, tag="sG")
            nc.sync.dma_start(sG[:], ag2[0:16, :])
            sGl = sb.tile([1, 8], f32, tag="sGl")
            nc.scalar.dma_start(sGl[:], ag2[16:17, :])
            tot0 = sb.tile([1, 1], f32, tag="tot0")
            nc.vector.tensor_reduce(tot0[:], sGl[:], mybir.AxisListType.X,
                                    OP.add)
            tot = sb.tile([1, 1], f32, tag="tot")
            nc.vector.tensor_scalar_add(tot[:], tot0[:], float(-CORR))

            # ---- levels 13..16 (replicated; ECS cols 14:22, 22:26, 26:28, 28)
            pYo, sYe = top_level(sG[:], 14, 8, c_u2Tf)  # f32 rhs from DMA
            sV = top_combine(pYo, sYe, 4, bf16)
            pYo, sYe = top_level(sV[:], 22, 4, c_u2T)
            sV = top_combine(pYo, sYe, 2, bf16)
            pYo, sYe = top_level(sV[:], 26, 2, c_u2T)
            sV = top_combine(pYo, sYe, 1, bf16)
            # root: unifurcating, left child only, no growth
            pYt, _ = top_level(sV[:], 28, 1, c_u2T)

            lnv = sb.tile([16, 1], f32, tag="lnv")
            nc.scalar.activation(lnv[:], pYt[:], AF.Ln)
            ptb = psN.tile([16, 1], f32, tag="N")
            nc.tensor.matmul(ptb[:], c_onesf, tot[:], start=True, stop=True)
            outv = sb.tile([16, 1], f32, tag="outv")
            nc.vector.tensor_add(outv[:], lnv[:], ptb[:])
            nc.sync.dma_start(out[:, :], outv[:])

    if split_waits:
        _split_multi_waits(nc)
    return nc


def _host_prep(branch_lens, init_partials, Q, growth_rates):
    bl = np.ascontiguousarray(np.asarray(branch_lens, dtype=F32))
    ip = np.asarray(init_partials, dtype=F32)
    Q64 = np.asarray(Q, dtype=np.float64)
    g64 = np.asarray(growth_rates, dtype=np.float64)
    R = Q64 - np.diag(g64)
    Wr, Winv, a, bsig, swap = _real_eig(R)
    Wsw = Wr[:, swap]

    I8 = np.eye(8)

    def bf(x):
        return np.asarray(x, dtype=np.float32).astype(BF16)

    packbf = np.zeros((128, PB_COLS), dtype=BF16)
    packbf[:, PB_WINV:PB_WINV + 128] = bf(np.kron(I8, Winv.T))
    packbf[:, PB_W:PB_W + 128] = bf(np.kron(I8, Wr.T))
    packbf[:, PB_WSW:PB_WSW + 128] = bf(np.kron(I8, Wsw.T))
    packbf[:, PB_ONESBD:PB_ONESBD + 128] = bf(np.kron(I8, np.ones((S, S))))
    packbf[:, PB_ONESC:PB_ONESC + 8] = bf(np.kron(I8, np.ones((S, 1))))
    packbf[:, PB_ITILE:PB_ITILE + 16] = bf(np.tile(np.eye(S), (8, 1)))
    packbf[0:16, PB_U2:PB_U2 + 32] = bf(np.hstack([Winv.T, Winv.T]))
    packbf[0:32, PB_Y2:PB_Y2 + 16] = bf(np.vstack([Wr.T, Wsw.T]))

    packf = np.zeros((128, PF_COLS), dtype=F32)
    packf[:, PF_AVEC] = np.tile(a, 8)
    packf[:, PF_IOTA] = np.arange(128) % 16
    packf[:, PF_GCOL] = np.tile(g64, 8)
    packf[0, PF_ONESF:PF_ONESF + 16] = 1.0
    packf[0:16, PF_U2F:PF_U2F + 32] = np.hstack([Winv.T, Winv.T])

    states = np.argmax(ip[:L], axis=1).astype(F32)

    base16 = np.zeros((8, P16_COLS), dtype=F16)
    base16[:, P16_EXPD:P16_EXPD + 128] = np.kron(I8, np.ones((1, S)))
    base16[:, P16_EXPB:P16_EXPB + 128] = np.kron(I8, bsig[None, :])

    consts = {"packbf": np.ascontiguousarray(packbf),
              "packf": np.ascontiguousarray(packf)}

    in_maps = []
    for c in range(NCORES):
        t_blk = np.zeros((8, T_ALL), dtype=F32)
        for hc in range(9):
            w = LPB >> hc
            base = OFFS[hc] + c * (LPC >> hc)
            seg = bl[base: base + (LPC >> hc)].reshape(8, w)
            t_blk[:, int(BLK_OFF[hc]): int(BLK_OFF[hc]) + w] = seg
        tt = np.concatenate([
            bl[OFFS[9] + c * 8: OFFS[9] + c * 8 + 8],
            bl[OFFS[10] + c * 4: OFFS[10] + c * 4 + 4],
            bl[OFFS[11] + c * 2: OFFS[11] + c * 2 + 2],
            bl[OFFS[12]: OFFS[12] + 8],
            bl[OFFS[13]: OFFS[13] + 4],
            bl[OFFS[14]: OFFS[14] + 2],
            bl[OFFS[15]: OFFS[15] + 1],
        ])
        t_blk[0, TOPO:TOPO + 29] = tt
        t_blk[1, TOPO:TOPO + 29] = tt
        p16 = base16.copy()
        p16[:, P16_T:P16_T + T_ALL] = t_blk.astype(F16)
        p16[:, P16_SID:P16_SID + 512] = \
            states[c * LPC:(c + 1) * LPC].reshape(8, 512).astype(F16)
        in_maps.append({"pack16": np.ascontiguousarray(p16), **consts})
    return in_maps


def kernel(postorder, children, parents, branch_lens, init_partials, Q,
           levels, growth_rates, *, _trace=False):
    in_maps = _host_prep(branch_lens, init_partials, Q, growth_rates)
    nc = build_nc()
    res = run_bass_kernel_spmd(nc, in_maps, core_ids=list(range(NCORES)),
                               trace=_trace)
    out = np.asarray(res.results[0]["out"], dtype=F32).reshape(S)
    if _trace:
        kernel.last_exec_time_ns = res.exec_time_ns
        kernel.last_results = res
    return out


# revision 28
# speedup vs baseline: 1.4333x; 1.4333x over previous
"""Trainium2 Bass kernel for Felsenstein pruning on a perfect binary tree
(nn_BaseLikelihoodModel: batched expm over edges + level-synchronous sweep).

Every edge matrix is expm(t_e * R) for ONE shared 16x16 rate matrix
R = Q - diag(growth).  A real block-eigendecomposition R = W M W^-1
(host-side, f64) turns the per-edge expm into per-edge exp/sin/cos factors
plus shared 16x16 matmuls:

    expm(t R) v = W * B(t) * (W^-1 v)

Key structure (v3):
  * swap-fold: Winv[swap] = P Winv and pairs share Re(lambda) with opposite
    Im signs, so swap_rows(ES) = -ES and the second matvec chain collapses:
    Y = W (EC * U) + W[:,swap] ((-ES) * U) with U = Winv V.
  * sin/cos via short polynomials on DVE/GpSimd (|b t| <= ~0.35); Scalar
    only ever needs the exp+ln activation table (no table thrash).
  * per-edge argument x = b*t is produced by a second PE broadcast with
    b-scaled selection weights; all inputs ship as ONE packed fp16 tensor
    (t is fp16: |a|*t*5e-4 per-edge log error, random-walks to ~2e-6).
  * leaves enter as state indices; one-hots built via broadcast + is_equal.
  * per level, the parent combine Ye*(g*Yo) splits the Y matmul into
    even(left)/odd(right) column groups so the PSUM->SBUF copy of Ye
    overlaps the odd-column matmuls.
  * stacked [Winv;Winv] / [W | W_swap] weights for the narrow top levels;
    levels 13..16 replicated on every core after a tiny AllGather.

The sweep runs in probability space with one per-node rescaling at level 8
(accumulated log-scales), mathematically identical to the reference's
log-space logsumexp recursion.  Edge factors carry a constant boost
e^BOOST; the exact total (N-1)*BOOST is subtracted at the end.
"""
import math
import numpy as np
import ml_dtypes

import concourse.bass as bass
import concourse.mybir as mybir
import concourse.tile as tile
from concourse.bass_utils import run_bass_kernel_spmd

F32 = np.float32
F16 = np.float16
BF16 = ml_dtypes.bfloat16
S = 16
L = 32768
N = 2 * L
NCORES = 8
NBLK = 8
LPC = L // NCORES          # 4096 leaves per core
LPB = LPC // NBLK          # 512 leaves per block
BLK_W = [LPB >> hc for hc in range(9)]             # 512..2
BLK_OFF = np.concatenate([[0], np.cumsum(BLK_W)])  # offsets into 1022
BLK_TOTAL = int(BLK_OFF[-1])                       # 1022
# top-edge columns TOPO..TOPO+28:
# [lvl9 x8][lvl10 x4][lvl11 x2][lvl12 x8][lvl13 x4][lvl14 x2][lvl15 x1]
TOPO = BLK_TOTAL                                   # 1022
T_ALL = 1056                                       # padded total columns
CHUNKS = [(0, 512), (512, 512), (1024, 32)]        # PSUM-bank sized chunks

OFFS = [0]
for _h in range(1, 16):
    OFFS.append(OFFS[-1] + (L >> (_h - 1)))
ROOT = N - 1

BOOST = 1.7
CORR = float(np.float64(N - 1) * np.float64(np.float32(BOOST)))
AGW = 32  # AllGather payload per core (16 state values + logscale + pad)

# pack16 column layout (fp16 [8, 1824])
P16_EXPD = 0      # kron(I8, ones(1,16))              [8, 0:128]
P16_EXPB = 128    # kron(I8, bsig row)                [8, 128:256]
P16_T = 256       # t_blk                             [8, 256:1312]
P16_SID = 1312    # leaf state ids                    [8, 1312:1824]
P16_COLS = 1824
# packbf column layout (bf16 [128, 584])
PB_WINV = 0      # kron(I8, Winv.T)            [128, 0:128]
PB_W = 128       # kron(I8, W.T)               [128, 128:256]
PB_WSW = 256     # kron(I8, W[:,swap].T)       [128, 256:384]
PB_ONESBD = 384  # kron(I8, ones(16,16))       [128, 384:512]
PB_ONESC = 512   # kron(I8, ones(16,1))        [128, 512:520]
PB_ITILE = 520   # tile(eye(16), (8,1))        [128, 520:536]
PB_U2 = 536      # [Winv.T | Winv.T]           [0:16, 536:568]
PB_Y2 = 568      # [W.T ; W[:,swap].T]         [0:32, 568:584]
PB_COLS = 584
# packf column layout (f32 [128, 52])
PF_AVEC = 0
PF_BVEC = 1
PF_IOTA = 2
PF_GCOL = 3
PF_ONESF = 4     # ones row-0                  [0:1, 4:20]
PF_U2F = 20      # [Winv.T | Winv.T] f32       [0:16, 20:52]
PF_COLS = 52

# sin/cos polynomial coefficients (|x| <= ~0.35); sine coeffs negated so the
# chain directly yields -sin(x) (the swap-fold sign).
NS5 = float(-1.0 / 120.0)
NS3 = float(1.0 / 6.0)
NS1 = float(-1.0)
C4 = float(1.0 / 24.0)
C2 = float(-0.5)


def _real_eig(R):
    """Real block eigendecomposition R = Wr @ M @ inv(Wr) with M block
    diagonal ([[a, b], [-b, a]] blocks for conjugate pairs)."""
    ev, V = np.linalg.eig(R)
    used = np.zeros(S, bool)
    order = np.argsort(-ev.real)
    cols = []
    for i in order:
        if used[i]:
            continue
        if abs(ev[i].imag) < 1e-12:
            cols.append(("real", i))
            used[i] = True
        else:
            j = None
            for i2 in order:
                if not used[i2] and i2 != i and abs(ev[i2] - ev[i].conj()) < 1e-8:
                    j = i2
                    break
            assert j is not None, "unpaired complex eigenvalue"
            ip = i if ev[i].imag > 0 else j
            cols.append(("pair", ip))
            used[i] = used[j] = True
    Wr = np.zeros((S, S))
    a = np.zeros(S)
    bsig = np.zeros(S)
    swap = np.arange(S)
    k = 0
    for c in cols:
        if c[0] == "real":
            i = c[1]
            Wr[:, k] = V[:, i].real
            a[k] = ev[i].real
            k += 1
        else:
            ip = c[1]
            lam = ev[ip]
            Wr[:, k] = V[:, ip].real
            Wr[:, k + 1] = V[:, ip].imag
            a[k] = a[k + 1] = lam.real
            bsig[k] = lam.imag
            bsig[k + 1] = -lam.imag
            swap[k] = k + 1
            swap[k + 1] = k
            k += 2
    assert k == S
    scales = np.ones(S)
    kk = 0
    while kk < S:
        if swap[kk] == kk:
            scales[kk] = np.linalg.norm(Wr[:, kk])
            kk += 1
        else:
            s = math.sqrt(np.linalg.norm(Wr[:, kk]) * np.linalg.norm(Wr[:, kk + 1]))
            scales[kk] = scales[kk + 1] = s
            kk += 2
    Wr = Wr / scales[None, :]
    Winv = np.linalg.inv(Wr)
    return Wr, Winv, a, bsig, swap


def _split_multi_waits(nc):
    """Walrus codegen allows only ONE sync-wait slot per engine instruction.
    Move extra waits onto prepended same-engine NoOps (queue order stalls
    identically)."""
    skip = (mybir.InstAllEngineBarrier, mybir.InstBranchHint,
            mybir.InstCompareAndBranch, mybir.InstUnconditionalBranch,
            mybir.InstIndirectBranch)
    for fn in nc.m.functions:
        for blk in fn.blocks:
            out = []
            for inst in blk.instructions:
                si = inst.sync_info
                if (si is not None and si.on_wait and len(si.on_wait) > 1
                        and not isinstance(inst, skip)):
                    waits = list(si.on_wait)
                    for i, w in enumerate(waits[:-1]):
                        nop = mybir.InstNoOp(
                            name=f"{inst.name}-wait{i}", ins=[], outs=[])
                        nop.engine = inst.engine
                        nop.sync_info = mybir.SyncInfo(
                            on_wait=[w], on_update=[])
                        out.append(nop)
                    inst.sync_info = mybir.SyncInfo(
                        on_wait=[waits[-1]], on_update=list(si.on_update or []))
                out.append(inst)
            blk.instructions = out


def build_nc(split_waits=True):
    f32 = mybir.dt.float32
    bf16 = mybir.dt.bfloat16
    f16 = mybir.dt.float16
    AF = mybir.ActivationFunctionType
    OP = mybir.AluOpType
    nc = bass.Bass()

    pack16 = nc.dram_tensor("pack16", [8, P16_COLS], f16, kind="ExternalInput")
    t128d = nc.dram_tensor("t128d", [128, T_ALL], f16, kind="ExternalInput")
    sid128d = nc.dram_tensor("sid128d", [128, 512], f16, kind="ExternalInput")
    packbf = nc.dram_tensor("packbf", [128, PB_COLS], bf16, kind="ExternalInput")
    packf = nc.dram_tensor("packf", [128, PF_COLS], f32, kind="ExternalInput")
    out = nc.dram_tensor("out", [16, 1], f32, kind="ExternalOutput")
    agin = nc.dram_tensor("agin", [AGW, 1], f32)
    agout = nc.dram_tensor("agout", [NCORES * AGW, 1], f32, addr_space="Shared")

    with tile.TileContext(nc) as tc:
        with (
            tc.tile_pool(name="const", bufs=1) as cp,
            tc.tile_pool(name="sb", bufs=2) as sb,
            tc.tile_pool(name="big", bufs=1) as bigp,
            tc.tile_pool(name="psU", bufs=2, space="PSUM") as psU,
            tc.tile_pool(name="psY", bufs=2, space="PSUM") as psY,
            tc.tile_pool(name="psN", bufs=1, space="PSUM") as psN,
        ):
            # ---- input DMAs, issued from different engines so the ~0.8us
            # DIRECT2D issue cost doesn't serialize on one sequencer
            s_p16 = cp.tile([8, P16_COLS], f16, tag="p16")
            nc.scalar.dma_start(s_p16[:], pack16[:, :])
            T128 = bigp.tile([128, T_ALL], f16, tag="T128")
            nc.scalar.dma_start(T128[:, 0:512], t128d[:, 0:512])
            nc.sync.dma_start(T128[:, 512:T_ALL], t128d[:, 512:T_ALL])
            SID128 = bigp.tile([128, 512], f16, tag="SID128")
            nc.gpsimd.dma_start(SID128[:], sid128d[:, :])
            s_pf = cp.tile([128, PF_COLS], f32, tag="packf")
            nc.gpsimd.dma_start(s_pf[:], packf[:, :])
            s_pb = cp.tile([128, PB_COLS], bf16, tag="packbf")
            nc.sync.dma_start(s_pb[:], packbf[:, :])

            v_expd = s_p16[:, P16_EXPD:P16_EXPD + 128]
            v_expB = s_p16[:, P16_EXPB:P16_EXPB + 128]
            v_t = s_p16[:, P16_T:P16_T + T_ALL]
            v_sid = s_p16[:, P16_SID:P16_SID + 512]
            c_winvT = s_pb[:, PB_WINV:PB_WINV + 128]
            c_wT = s_pb[:, PB_W:PB_W + 128]
            c_wswT = s_pb[:, PB_WSW:PB_WSW + 128]
            c_onesbd = s_pb[:, PB_ONESBD:PB_ONESBD + 128]
            c_onesc = s_pb[:, PB_ONESC:PB_ONESC + 8]
            c_itile = s_pb[:, PB_ITILE:PB_ITILE + 16]
            c_u2T = s_pb[0:16, PB_U2:PB_U2 + 32]
            c_y2T = s_pb[0:32, PB_Y2:PB_Y2 + 16]
            c_avec = s_pf[:, PF_AVEC:PF_AVEC + 1]
            c_bvec = s_pf[:, PF_BVEC:PF_BVEC + 1]
            c_iota = s_pf[:, PF_IOTA:PF_IOTA + 1]
            c_gcol = s_pf[:, PF_GCOL:PF_GCOL + 1]
            c_onesf = s_pf[0:1, PF_ONESF:PF_ONESF + 16]
            c_u2Tf = s_pf[0:16, PF_U2F:PF_U2F + 32]

            cBOOST = cp.tile([128, 1], f32, tag="boost")
            nc.vector.memset(cBOOST[:], float(BOOST))
            ones8 = cp.tile([8, 1], f32, tag="ones8")
            nc.vector.memset(ones8[:], 1.0)
            ls12 = cp.tile([16, 1], f32, tag="ls12")
            nc.vector.memset(ls12[:], 0.0)
            # dummy activation with no data dependency: forces the exp+ln
            # ACT table load at engine boot instead of on the critical path
            dummy = cp.tile([1, 1], f32, tag="dummy")
            nc.scalar.activation(dummy[:], cBOOST[0:1, 0:1], AF.Exp)

            # ---- PE queue-observer preamble (one per DMA'd matmul operand)
            pobs = psY.tile([1, 1], f32, tag="Ye")
            nc.tensor.matmul(pobs[:], s_p16[0:1, 0:1], s_p16[0:1, 0:1],
                             start=True, stop=True)
            pobs2 = psN.tile([1, 1], f32, tag="N")
            nc.tensor.matmul(pobs2[:], s_pb[0:1, 0:1], s_pb[0:1, 0:1],
                             start=True, stop=True)
            pobs3 = psN.tile([1, 1], f32, tag="N2")
            nc.tensor.matmul(pobs3[:], s_pf[0:1, 0:1], s_pf[0:1, 0:1],
                             start=True, stop=True)

            # ---- leaf one-hots: compare pre-broadcast state ids to row state
            sX = bigp.tile([128, 512], bf16, tag="V0")
            nc.vector.tensor_scalar(sX[:], SID128[:], c_iota, None, OP.is_equal)

            # ---- edge factors.  Per chunk: T128 = t broadcast (PE), x = b*t
            # broadcast (PE, b-scaled weights); E = exp(a t + BOOST) and
            # x^2 (Square) on Scalar; -sin/cos chains in bf16 on DVE/GpSimd.
            # EC and ESp live in ONE tile so the per-level m12 multiply can
            # read [EC_lvl ; ESp_lvl] as a single strided AP.
            ECES = bigp.tile([128, 2 * T_ALL], bf16, tag="ECES")
            EC = ECES[:, 0:T_ALL]
            ESp = ECES[:, T_ALL:2 * T_ALL]
            sE = bigp.tile([128, T_ALL], bf16, tag="sE")
            for lo, wch in CHUNKS:
                ts_ = T128[:, lo:lo + wch]
                nc.scalar.activation(sE[:, lo:lo + wch], ts_, AF.Exp,
                                     bias=cBOOST[:, 0:1], scale=c_avec)
                x2 = sb.tile([128, wch], bf16, tag="x2")
                nc.scalar.activation(x2[:], ts_, AF.Square, scale=c_bvec)
                # 2-term -sin: (x2/6 - 1) * (b t)   (err x^5/120)
                q = sb.tile([128, wch], bf16, tag="q")
                nc.vector.tensor_scalar(q[:], x2[:], float(1.0 / 6.0), -1.0,
                                        OP.mult, OP.add)
                r = sb.tile([128, wch], bf16, tag="r")
                nc.vector.scalar_tensor_tensor(r[:], ts_, c_bvec, q[:],
                                               OP.mult, OP.mult)
                if lo == 0:
                    nc.vector.tensor_mul(ESp[:, lo:lo + wch], r[:],
                                         sE[:, lo:lo + wch])
                else:
                    nc.gpsimd.tensor_mul(ESp[:, lo:lo + wch], r[:],
                                         sE[:, lo:lo + wch])
                # 2-term cos * E: E + (x2 * -0.5) * E   (err x^4/24)
                p = sb.tile([128, wch], bf16, tag="p")
                nc.vector.scalar_tensor_tensor(
                    p[:], x2[:], -0.5, sE[:, lo:lo + wch], OP.mult, OP.mult)
                if lo == 0:
                    nc.vector.tensor_add(EC[:, lo:lo + wch],
                                         sE[:, lo:lo + wch], p[:])
                else:
                    nc.gpsimd.tensor_add(EC[:, lo:lo + wch],
                                         sE[:, lo:lo + wch], p[:])
            # stacked top-edge factors [EC; ESp] (both from block 0); the
            # SBUF->SBUF DMA sidesteps the 32-aligned partition-base rule
            ECS = sb.tile([32, 29], bf16, tag="ECS")
            nc.sync.dma_start(ECS[0:16, :], EC[0:16, TOPO:TOPO + 29])
            nc.sync.dma_start(ECS[16:32, :], ESp[0:16, TOPO:TOPO + 29])

            # ---- one sweep level: U = Winv V; m12 = [EC_lvl|ESp_lvl] * [U|U]
            # (one DVE op via a 3-dim AP into ECES); Ye/Yo = even/odd column
            # groups of W m1 + Wsw m2 so the Ye copy overlaps the odd
            # matmuls; parent = Ye * (g * Yo).
            ecs3 = ECES[:].rearrange("p (k w) -> p k w", k=2)

            def level(V, lo, wc, h):
                wp = wc // 2
                if wc > 256:
                    # 2*wc f32 would cross a PSUM bank; use the 2-op form
                    pU = psU.tile([128, wc], f32, tag="U")
                    nc.tensor.matmul(pU[:], c_winvT, V, start=True, stop=True)
                    m12 = sb.tile([128, 2 * wc], bf16, tag="m12w")
                    nc.vector.tensor_mul(m12[:, 0:wc], EC[:, lo:lo + wc], pU[:])
                    nc.vector.tensor_mul(m12[:, wc:2 * wc],
                                         ESp[:, lo:lo + wc], pU[:])
                else:
                    pU = psU.tile([128, 2 * wc], f32, tag="U")
                    nc.tensor.matmul(pU[:, 0:wc], c_winvT, V,
                                     start=True, stop=True)
                    nc.tensor.matmul(pU[:, wc:2 * wc], c_winvT, V,
                                     start=True, stop=True)
                    m12 = sb.tile([128, 2 * wc], bf16, tag="m12w")
                    nc.vector.tensor_mul(
                        m12[:].rearrange("p (k w) -> p k w", k=2),
                        ecs3[:, :, lo:lo + wc],
                        pU[:].rearrange("p (k w) -> p k w", k=2))
                pYe = psY.tile([128, wp], f32, tag="Ye")
                nc.tensor.matmul(pYe[:], c_wT, m12[:, 0:wc:2],
                                 start=True, stop=False)
                nc.tensor.matmul(pYe[:], c_wswT, m12[:, wc:2 * wc:2],
                                 start=False, stop=True)
                sYe = sb.tile([128, wp], f32, tag="sYe")
                if h <= 2:
                    nc.scalar.copy(sYe[:], pYe[:])
                else:
                    nc.vector.tensor_copy(sYe[:], pYe[:])
                pYo = psY.tile([128, wp], f32, tag="Yo")
                nc.tensor.matmul(pYo[:], c_wT, m12[:, 1:wc:2],
                                 start=True, stop=False)
                nc.tensor.matmul(pYo[:], c_wswT, m12[:, wc + 1:2 * wc:2],
                                 start=False, stop=True)
                return pYo, sYe

            # ---- level sweep 1..9 (8 blocks x 16 states on 128 partitions)
            V = sX
            lsW = None
            for h in range(1, 10):
                wc = BLK_W[h - 1]
                lo = int(BLK_OFF[h - 1])
                wp = wc // 2
                pYo, sYe = level(V[:], lo, wc, h)
                if h == 8:
                    praw = sb.tile([128, wp], bf16, tag="Vc")
                    nc.vector.scalar_tensor_tensor(
                        praw[:], pYo[:], c_gcol, sYe[:], OP.mult, OP.mult)
                    pSb = psN.tile([128, wp], f32, tag="N")
                    nc.tensor.matmul(pSb[:], c_onesbd, praw[:],
                                     start=True, stop=True)
                    pSc = psN.tile([8, wp], f32, tag="N2")
                    nc.tensor.matmul(pSc[:], c_onesc, praw[:],
                                     start=True, stop=True)
                    rb = sb.tile([128, wp], f32, tag="rb")
                    nc.vector.reciprocal(rb[:], pSb[:])
                    Vn = sb.tile([128, wp], bf16, tag="V")
                    nc.vector.tensor_mul(Vn[:], praw[:], rb[:])
                    lnS = sb.tile([8, wp], f32, tag="lnS")
                    nc.scalar.activation(lnS[:], pSc[:], AF.Ln)
                    lsW = lnS
                else:
                    Vn = sb.tile([128, wp], f32 if h == 9 else bf16, tag="V")
                    nc.vector.scalar_tensor_tensor(
                        Vn[:], pYo[:], c_gcol, sYe[:], OP.mult, OP.mult)
                V = Vn

            # per-core log-scale total
            ls9 = sb.tile([8, 1], f32, tag="ls9")
            nc.gpsimd.tensor_add(ls9[:], lsW[:, 0:1], lsW[:, 1:2])
            pls = psN.tile([1, 1], f32, tag="N2")
            nc.tensor.matmul(pls[:], ls9[:], ones8[:], start=True, stop=True)
            nc.vector.tensor_copy(ls12[0:1, :], pls[:])

            # ---- reshape core state to single block: V (128x1) -> (16x8)
            rhs8 = sb.tile([128, 8], bf16, tag="rhs8")
            nc.vector.tensor_scalar_mul(rhs8[:], c_onesc, V[:, 0:1])
            pV9 = psY.tile([16, 8], f32, tag="Ye")
            nc.tensor.matmul(pV9[:], c_itile, rhs8[:], start=True, stop=True)
            sV = sb.tile([16, 8], bf16, tag="sV")
            nc.vector.tensor_copy(sV[:], pV9[:])

            # ---- one stacked top level (32 partitions)
            def top_level(rhsV, off, n, lhsU):
                pU2 = psU.tile([32, n], f32, tag="U")
                nc.tensor.matmul(pU2[:], lhsU, rhsV, start=True, stop=True)
                m12 = sb.tile([32, n], bf16, tag="m12")
                nc.vector.tensor_mul(m12[:], ECS[:, off:off + n], pU2[:])
                if n == 1:
                    pYt = psY.tile([16, 1], f32, tag="Ye")
                    nc.tensor.matmul(pYt[:], c_y2T, m12[:], start=True, stop=True)
                    return pYt, None
                pYe = psY.tile([16, n // 2], f32, tag="Ye")
                nc.tensor.matmul(pYe[:], c_y2T, m12[:, 0::2], start=True, stop=True)
                sYe = sb.tile([16, n // 2], f32, tag="sYe")
                nc.vector.tensor_copy(sYe[:], pYe[:])
                pYo = psY.tile([16, n // 2], f32, tag="Yo")
                nc.tensor.matmul(pYo[:], c_y2T, m12[:, 1::2], start=True, stop=True)
                return pYo, sYe

            def top_combine(pYo, sYe, n2, out_dt, out_ap=None):
                if out_ap is None:
                    Vn = sb.tile([16, n2], out_dt, tag="sV")
                    out_ap = Vn[:]
                else:
                    Vn = None
                nc.vector.scalar_tensor_tensor(
                    out_ap, pYo[:], c_gcol[0:16, 0:1], sYe[:],
                    OP.mult, OP.mult)
                return Vn

            # levels 10..12 (within-core top; ECS cols 0:8, 8:12, 12:14)
            off = 0
            n = 8
            for h in (10, 11, 12):
                pYo, sYe = top_level(sV[:], off, n, c_u2T)
                off += n
                n //= 2
                if h == 12:
                    sV12 = sb.tile([16, 1], f32, tag="sV12")
                    top_combine(pYo, sYe, 1, f32, out_ap=sV12[:])
                else:
                    sV = top_combine(pYo, sYe, n, bf16)

            # ---- AllGather of (16-vec, logscale) across the 8 cores
            nc.sync.dma_start(agin[0:16, 0:1], sV12[:])
            nc.sync.dma_start(agin[16:32, 0:1], ls12[:])
            nc.gpsimd.collective_compute(
                "AllGather",
                OP.bypass,
                replica_groups=[list(range(NCORES))],
                ins=[agin[:, :].opt()],
                outs=[agout[:, :].opt()],
            )
            ag2 = agout[:, 0].rearrange("(r v) -> v r", v=AGW)
            sG = sb.tile([16, 8], f32, tag="sG")
            nc.sync.dma_start(sG[:], ag2[0:16, :])
            sGl = sb.tile([1, 8], f32, tag="sGl")
            nc.scalar.dma_start(sGl[:], ag2[16:17, :])
            tot0 = sb.tile([1, 1], f32, tag="tot0")
            nc.vector.tensor_reduce(tot0[:], sGl[:], mybir.AxisListType.X,
                                    OP.add)
            tot = sb.tile([1, 1], f32, tag="tot")
            nc.vector.tensor_scalar_add(tot[:], tot0[:], float(-CORR))

            # ---- levels 13..16 (replicated; ECS cols 14:22, 22:26, 26:28, 28)
            pYo, sYe = top_level(sG[:], 14, 8, c_u2Tf)  # f32 rhs from DMA
            sV = top_combine(pYo, sYe, 4, bf16)
            pYo, sYe = top_level(sV[:], 22, 4, c_u2T)
            sV = top_combine(pYo, sYe, 2, bf16)
            pYo, sYe = top_level(sV[:], 26, 2, c_u2T)
            sV = top_combine(pYo, sYe, 1, bf16)
            # root: unifurcating, left child only, no growth
            pYt, _ = top_level(sV[:], 28, 1, c_u2T)

            lnv = sb.tile([16, 1], f32, tag="lnv")
            nc.scalar.activation(lnv[:], pYt[:], AF.Ln)
            ptb = psN.tile([16, 1], f32, tag="N")
            nc.tensor.matmul(ptb[:], c_onesf, tot[:], start=True, stop=True)
            outv = sb.tile([16, 1], f32, tag="outv")
            nc.vector.tensor_add(outv[:], lnv[:], ptb[:])
            nc.sync.dma_start(out[:, :], outv[:])

    if split_waits:
        _split_multi_waits(nc)
    return nc


def _host_prep(branch_lens, init_partials, Q, growth_rates):
    bl = np.ascontiguousarray(np.asarray(branch_lens, dtype=F32))
    ip = np.asarray(init_partials, dtype=F32)
    Q64 = np.asarray(Q, dtype=np.float64)
    g64 = np.asarray(growth_rates, dtype=np.float64)
    R = Q64 - np.diag(g64)
    Wr, Winv, a, bsig, swap = _real_eig(R)
    Wsw = Wr[:, swap]

    I8 = np.eye(8)

    def bf(x):
        return np.asarray(x, dtype=np.float32).astype(BF16)

    packbf = np.zeros((128, PB_COLS), dtype=BF16)
    packbf[:, PB_WINV:PB_WINV + 128] = bf(np.kron(I8, Winv.T))
    packbf[:, PB_W:PB_W + 128] = bf(np.kron(I8, Wr.T))
    packbf[:, PB_WSW:PB_WSW + 128] = bf(np.kron(I8, Wsw.T))
    packbf[:, PB_ONESBD:PB_ONESBD + 128] = bf(np.kron(I8, np.ones((S, S))))
    packbf[:, PB_ONESC:PB_ONESC + 8] = bf(np.kron(I8, np.ones((S, 1))))
    packbf[:, PB_ITILE:PB_ITILE + 16] = bf(np.tile(np.eye(S), (8, 1)))
    packbf[0:16, PB_U2:PB_U2 + 32] = bf(np.hstack([Winv.T, Winv.T]))
    packbf[0:32, PB_Y2:PB_Y2 + 16] = bf(np.vstack([Wr.T, Wsw.T]))

    packf = np.zeros((128, PF_COLS), dtype=F32)
    packf[:, PF_AVEC] = np.tile(a, 8)
    packf[:, PF_BVEC] = np.tile(bsig, 8)
    packf[:, PF_IOTA] = np.arange(128) % 16
    packf[:, PF_GCOL] = np.tile(g64, 8)
    packf[0, PF_ONESF:PF_ONESF + 16] = 1.0
    packf[0:16, PF_U2F:PF_U2F + 32] = np.hstack([Winv.T, Winv.T])

    states = np.argmax(ip[:L], axis=1).astype(F32)

    base16 = np.zeros((8, P16_COLS), dtype=F16)
    base16[:, P16_EXPD:P16_EXPD + 128] = np.kron(I8, np.ones((1, S)))
    base16[:, P16_EXPB:P16_EXPB + 128] = np.kron(I8, bsig[None, :])

    consts = {"packbf": np.ascontiguousarray(packbf),
              "packf": np.ascontiguousarray(packf)}

    in_maps = []
    for c in range(NCORES):
        t_blk = np.zeros((8, T_ALL), dtype=F32)
        for hc in range(9):
            w = LPB >> hc
            base = OFFS[hc] + c * (LPC >> hc)
            seg = bl[base: base + (LPC >> hc)].reshape(8, w)
            t_blk[:, int(BLK_OFF[hc]): int(BLK_OFF[hc]) + w] = seg
        tt = np.concatenate([
            bl[OFFS[9] + c * 8: OFFS[9] + c * 8 + 8],
            bl[OFFS[10] + c * 4: OFFS[10] + c * 4 + 4],
            bl[OFFS[11] + c * 2: OFFS[11] + c * 2 + 2],
            bl[OFFS[12]: OFFS[12] + 8],
            bl[OFFS[13]: OFFS[13] + 4],
            bl[OFFS[14]: OFFS[14] + 2],
            bl[OFFS[15]: OFFS[15] + 1],
        ])
        t_blk[0, TOPO:TOPO + 29] = tt
        t_blk[1, TOPO:TOPO + 29] = tt
        p16 = base16.copy()
        p16[:, P16_T:P16_T + T_ALL] = t_blk.astype(F16)
        p16[:, P16_SID:P16_SID + 512] = \
            states[c * LPC:(c + 1) * LPC].reshape(8, 512).astype(F16)
        t128 = np.ascontiguousarray(np.repeat(t_blk.astype(F16), 16, axis=0))
        sid128 = np.ascontiguousarray(np.repeat(
            states[c * LPC:(c + 1) * LPC].reshape(8, 512).astype(F16),
            16, axis=0))
        in_maps.append({"pack16": np.ascontiguousarray(p16),
                        "t128d": t128, "sid128d": sid128, **consts})
    return in_maps


def kernel(postorder, children, parents, branch_lens, init_partials, Q,
           levels, growth_rates, *, _trace=False):
    in_maps = _host_prep(branch_lens, init_partials, Q, growth_rates)
    nc = build_nc()
    res = run_bass_kernel_spmd(nc, in_maps, core_ids=list(range(NCORES)),
                               trace=_trace)
    out = np.asarray(res.results[0]["out"], dtype=F32).reshape(S)
    if _trace:
        kernel.last_exec_time_ns = res.exec_time_ns
        kernel.last_results = res
    return out
